# revision 33
# baseline (speedup 1.0000x reference)
"""Trainium2 Bass kernel for nn_ApplicationScoringLayer (v3).

Computes out[l, r] = log_softmax(ts+bias, vocab)[:, lidx[l]] log-matmul-exp
log_softmax(rts[ridx], types) -> [L, R] via the restructuring:

    sa[r, k]  = softmax(rts[ridx[r], :])[k]         (row softmax over K=64)
    eb[k, l]  = exp(ts[k, lidx[l]]) / Z_k           (bias factored OUT)
    Z_k       = sum_i exp(ts[k, i] + bias[i])       (full-vocab row sum)
    out[l, r] = ln( e^{bias[lidx[l]]} * sum_k eb[k, l] * sa[r, k] )

The bias enters eb multiplicatively per OUTPUT ROW l, so it folds into the
final Ln as a per-partition activation `scale` (ln(scale*x)).

Sharding: 4x2 grid over the output. Core c = (i, j) = (c//2, c%2) computes
out[i*2048:(i+1)*2048, j*4096:(j+1)*4096]:
  - gathers use GPSIMD dma_gather (ONE instruction per call: ~1us + 0.34ns/row
    vs ~1us per 128 rows for indirect DMA). int16 index limit -> the vocab is
    split into 4 ranges of 32768 rows; the host ships per-range clamped int16
    index tiles + 0/1 masks, the device gathers each range and merges with 3
    predicated copies. Extra HBM read traffic (4x) is cheap vs. queue time.
  - A side gathers all 4096 ridx rows of rts; B side its 2048 lidx rows of
    tsb2 (transposed table, 128-wide rows for the 256B-multiple elem rule:
    cols 0:64 = ts.T, col 64 = bias).
  - Z pass: vocab/8 shard per core, ONE flat contiguous DMA stream
    (12544x65 -> [128, 6370]), bias-add via free-axis broadcast, exp->bf16,
    partition-reduced by 13 wide accumulating PE matmuls against ones
    (8 vocab rows per matmul; tiny matmuls would pay ~200ns+ fixed cost
    each), tree-merged on psum row 0. Partials cross-core summed via a tiny
    [1,64] AllGather + local sum; eb multiplies by 1/Z (reciprocal, not
    exp(-lnZ): avoids flipping the ACT function table between Exp and Ln).
  - Output computed in bf16 (tolerance 2e-2; bf16 end-to-end ~4e-3), stored
    bf16 [2048, 4096] per core, host concatenates + upcasts. Halves the
    dominant store traffic.

Scheduling notes: the Tile scheduler orders by readiness estimates, so the
Z-critical path is priority-0 and explicitly pinned ahead of the A-side
stream per engine with nosync edges; the AllGather dispatches after the B
gather batch (its SEQ-head wait then lines up with its input landing);
B-side transposes/exps are deferred past zrec so the transpose-psum ring
is never held across the collective. Measured: 56.2us/core HW exec
(baseline staged kernel: 317us).
"""
import os
import sys

for _p in ("/opt/trn_rl_repo", os.path.expanduser("~/.axon_site/_ro/trn_rl_repo")):
    if os.path.isdir(_p) and _p not in sys.path:
        sys.path.insert(0, _p)

import contextlib

import numpy as np

import concourse.bacc as bacc
import concourse.bass as bass
import concourse.tile as tile
from concourse import mybir
from concourse.bass_utils import run_bass_kernel_spmd
from concourse.masks import make_identity
from concourse.tile import add_dep_helper

F32 = mybir.dt.float32
BF16 = mybir.dt.bfloat16
I16 = mybir.dt.int16
I8 = mybir.dt.int8
AF = mybir.ActivationFunctionType
ALU = mybir.AluOpType
AX = mybir.AxisListType

V = 100000    # vocab size (both tables)
K = 64        # num types
R = 8192      # num rhs idxs
L = 8192      # num lhs idxs
N_CORES = 8
PI, PJ = 4, 2            # output grid: l split x r split
LS = L // PI             # 2048 output rows per core
RS = R // PJ             # 4096 output cols per core
VP = 100352              # vocab padded (pad rows: ts=-50, bias=0 -> exp ~ 0)
ZR = VP // N_CORES       # 12544 Z-pass rows per core
ZF = ZR * 65 // 128      # 6370 flat f32 per partition
GA = RS // 128           # 32 A-side gather groups
GB = LS // 128           # 16 B-side gather groups
NR = 4                   # int16 index ranges
RW = 32768               # range width
ZROWS = (24, 24, 24, 26)  # Z chunk rows/partition (8-aligned starts)


def build(v=V, k=K, r=R, l=L, n_cores=N_CORES, repeat=1, loop=1):
    """Build the SPMD Bass program (same NEFF on all cores)."""
    assert (v, k, r, l, n_cores) == (V, K, R, L, N_CORES)
    nc = bacc.Bacc("TRN2", target_bir_lowering=False, debug=False,
                   num_devices=n_cores)

    rts = nc.dram_tensor("rts", [V, K], F32, kind="ExternalInput")
    # transposed table, 128 f32/row: cols 0:64 = ts.T, col 64 = bias
    tsb2 = nc.dram_tensor("tsb2", [VP, 128], F32, kind="ExternalInput")
    # this core's Z shard (65-wide rows, host-sliced)
    zts = nc.dram_tensor("zts", [ZR, 65], F32, kind="ExternalInput")
    # per-range clamped int16 idxs, 16-row blocks replicated x8 down partitions
    aidx = nc.dram_tensor("aidx", [128, NR * (RS // 16)], I16,
                          kind="ExternalInput")
    bidx = nc.dram_tensor("bidx", [128, NR * (LS // 16)], I16,
                          kind="ExternalInput")
    # 0/1 range-membership masks for ranges 1..3, expanded per element so
    # the copy_predicated mask stream matches the data stream exactly
    amask = nc.dram_tensor("amask", [128, (NR - 1) * GA * K], I8,
                           kind="ExternalInput")
    bmask = nc.dram_tensor("bmask", [128, (NR - 1) * GB * 128], I8,
                           kind="ExternalInput")
    out = nc.dram_tensor("out", [LS, RS], BF16, kind="ExternalOutput")

    groups = [list(range(n_cores))]
    AW = RS // 16            # 256 idx cols per A range
    BW = LS // 16            # 128 idx cols per B range

    with tile.TileContext(nc) as tc:
        with (
            tc.tile_pool(name="persist", bufs=1) as pp,
            tc.tile_pool(name="zstream", bufs=2) as zp,
            tc.tile_pool(name="ostage", bufs=3) as op_,
            tc.tile_pool(name="psmm", bufs=2, space="PSUM") as ps_mm,
            tc.tile_pool(name="pstr", bufs=2, space="PSUM") as ps_tr,
            tc.tile_pool(name="psz", bufs=1, space="PSUM") as ps_z,
            tc.tile_pool(name="dram", bufs=1, space="DRAM") as dp,
        ):
            identB = pp.tile([128, 128], BF16, tag="identB")
            make_identity(nc, identB[:])
            ones = pp.tile([128, 64], BF16, tag="ones")
            nc.vector.memset(ones[:], 1.0)
            aidx_sb = pp.tile([128, NR * AW], I16, tag="aidx")
            nc.sync.dma_start(aidx_sb[:], aidx[:])
            bidx_sb = pp.tile([128, NR * BW], I16, tag="bidx")
            nc.sync.dma_start(bidx_sb[:], bidx[:])
            amask_sb = pp.tile([128, (NR - 1) * GA * K], I8, tag="amask")
            nc.sync.dma_start(amask_sb[:], amask[:])
            bmask_sb = pp.tile([128, (NR - 1) * GB * 128], I8, tag="bmask")
            nc.sync.dma_start(bmask_sb[:], bmask[:])

            loop_ctx = tc.For_i(0, loop, 1) if loop > 1 else contextlib.nullcontext()
            with loop_ctx:
              for _rep in range(repeat):
                eab = pp.tile([128, GA * K], BF16, tag="eab")
                rsum = pp.tile([128, GA], F32, tag="rsum")
                rrec = pp.tile([128, GA], F32, tag="rrec")
                saT = pp.tile([64, RS], BF16, tag="saT")
                ga = [pp.tile([128, GA * K], F32, tag=f"ga{t}", name=f"ga{t}")
                      for t in range(NR)]
                gb = [pp.tile([128, GB * 128], F32, tag=f"gb{t}", name=f"gb{t}")
                      for t in range(NR)]
                ea = ga[0]            # range-0 tile doubles as the A acc

                # ---- Z pass at scheduler priority 0 (critical path to the
                # AllGather) ------------------------------------------------
                with tc.high_priority():
                    zps = ps_z.tile([64, 512], F32, tag="zps")
                    nz = sum(ZROWS)
                    col = 0
                    ri = 0
                    zadd = zmm = None
                    for nr in ZROWS:
                        zt = zp.tile([128, max(ZROWS) * 65], F32, tag="zt",
                                     bufs=3)
                        nc.sync.dma_start(
                            zt[:, 0:nr * 65],
                            bass.AP(zts, col, [[ZF, 128], [1, nr * 65]]))
                        zt3 = zt[:, 0:nr * 65].rearrange("p (g c) -> p g c",
                                                         c=65)
                        zb = zp.tile([128, max(ZROWS) * 64], F32, tag="zb",
                                     bufs=3)
                        zadd = nc.vector.tensor_tensor(
                            out=zb[:, 0:nr * 64].rearrange("p (g c) -> p g c",
                                                           c=64),
                            in0=zt3[:, :, 0:64],
                            in1=zt3[:, :, 64:65].to_broadcast([128, nr, 64]),
                            op=ALU.add)
                        ze = zp.tile([128, max(ZROWS) * 64], BF16, tag="ze",
                                     bufs=3)
                        zexp = nc.scalar.activation(ze[:, 0:nr * 64],
                                                    zb[:, 0:nr * 64], AF.Exp)
                        for t in range(0, nr, 8):
                            w = min(8, nr - t) * 64
                            zmm = nc.tensor.matmul(
                                zps[:, 0:w], lhsT=ones[:],
                                rhs=ze[:, t * 64:t * 64 + w],
                                start=(ri == 0), stop=(ri + 8 >= nz),
                                skip_group_check=True)
                            ri += 8
                        col += nr * 65

                    # tree-merge the 8 sub-sums on psum row 0 (all rows of a
                    # ones-matmul are identical) -> zin [1, 64] -> DRAM
                    zm0 = pp.tile([1, 256], F32, tag="zm0")
                    nc.vector.tensor_copy(zm0[:], zps[0:1, 0:256])
                    zm1 = pp.tile([1, 256], F32, tag="zm1")
                    nc.vector.tensor_tensor(out=zm1[:], in0=zm0[:],
                                            in1=zps[0:1, 256:512], op=ALU.add)
                    zm2 = pp.tile([1, 128], F32, tag="zm2")
                    nc.vector.tensor_tensor(out=zm2[:], in0=zm1[:, 0:128],
                                            in1=zm1[:, 128:256], op=ALU.add)
                    zin = pp.tile([1, 64], F32, tag="zinsb")
                    zcopy = nc.vector.tensor_tensor(
                        out=zin[:], in0=zm2[:, 0:64], in1=zm2[:, 64:128],
                        op=ALU.add)
                    zin_d = dp.tile([1, 64], F32, tag="zin")
                    nc.sync.dma_start(zin_d[:], zin[:])

                # ---- gather stream (gpsimd queue, 4 SWDGE queues): each
                # dma_gather is <=1024 idxs (the SWDGE ring holds 1024
                # descriptors), round-robined over the queues. Plain
                # (self-triggered) calls with NO manual completion sem: the
                # Tile sem pass attaches its own DMASW lane semaphore as
                # OnUpdate[0] (a manual sem= would occupy that slot and
                # deadlock every consumer). Each range reads the table AT
                # ITS BASE OFFSET (idxs are range-relative int16).
                def gather_chunk(dst, dsl, tbl_t, nrows, elem, isl):
                    nc.gpsimd.dma_gather(
                        out_ap=dst[:, dsl].rearrange("p (c e) -> p c e",
                                                     e=elem),
                        in_ap=tbl_t, idxs_ap=isl,
                        num_idxs=1024, num_idxs_reg=1024, elem_size=elem)

                # B side first (2 quads of 4 ranges), eager merge + prep
                ebias = pp.tile([128, GB], F32, tag="ebias")
                tsbB = pp.tile([128, GB * K], BF16, tag="tsbB")
                ebp = pp.tile([64, LS], BF16, tag="ebp")
                t3 = gb[0][:].rearrange("p (g c) -> p g c", c=128)
                for hb in range(2):
                    gsl = slice(hb * 8 * 128, (hb + 1) * 8 * 128)
                    for t in range(NR):
                        gather_chunk(
                            gb[t], gsl,
                            bass.AP(tsb2, t * RW * 128,
                                    [[128, VP - t * RW], [1, 128]]),
                            VP - t * RW, 128,
                            bidx_sb[:, t * BW + hb * 64:
                                    t * BW + (hb + 1) * 64])
                    for t in range(1, NR):
                        nc.vector.copy_predicated(
                            out=gb[0][:, gsl],
                            mask=bmask_sb[:, (t - 1) * GB * 128 + hb * 8 * 128:
                                          (t - 1) * GB * 128 +
                                          (hb + 1) * 8 * 128],
                            data=gb[t][:, gsl])
                    csl = slice(hb * 8, (hb + 1) * 8)
                    nc.scalar.activation(
                        ebias[:, csl].rearrange("p (g o) -> p g o", o=1),
                        t3[:, csl, 64:65], AF.Exp)
                    nc.vector.tensor_copy(
                        tsbB[:, hb * 8 * K:(hb + 1) * 8 * K].rearrange(
                            "p (g c) -> p g c", c=K),
                        t3[:, csl, 0:64])

                # the Z AllGather dispatches behind the B gathers; its input
                # lands at about the time the queue drains to it
                zall_d = dp.tile([n_cores, 1, 64], F32, tag="zall")
                nc.gpsimd.collective_compute(
                    "AllGather", ALU.bypass, replica_groups=groups,
                    ins=[zin_d[:]], outs=[zall_d[:]])

                def a_cluster(c, after=()):
                    """exp -> row softmax -> bf16 -> transpose, groups 4c..4c+3
                    -> saT columns [512c, 512c+512)."""
                    sl = slice(c * 4 * K, (c + 1) * 4 * K)
                    aexp = nc.scalar.activation(ea[:, sl], ea[:, sl], AF.Exp)
                    e3 = ea[:, sl].rearrange("p (g c) -> p g c", c=K)
                    ared = nc.vector.reduce_sum(rsum[:, c * 4:(c + 1) * 4],
                                                e3, axis=AX.X)
                    nc.vector.reciprocal(rrec[:, c * 4:(c + 1) * 4],
                                         rsum[:, c * 4:(c + 1) * 4])
                    nc.vector.tensor_tensor(
                        out=eab[:, sl].rearrange("p (g c) -> p g c", c=K),
                        in0=e3,
                        in1=rrec[:, c * 4:(c + 1) * 4].rearrange(
                            "p (g o) -> p g o", o=1).to_broadcast([128, 4, K]),
                        op=ALU.mult)
                    pst = ps_tr.tile([64, 512], BF16, tag="tr")
                    atr = None
                    for t in range(4):
                        tr = nc.tensor.transpose(
                            out=pst[:, t * 128:(t + 1) * 128],
                            in_=eab[:, (c * 4 + t) * K:(c * 4 + t + 1) * K],
                            identity=identB[:])
                        atr = atr or tr
                    nc.vector.tensor_copy(saT[:, c * 512:(c + 1) * 512],
                                          pst[:])
                    for first, dep in zip((aexp, ared, atr), after):
                        if dep is not None:
                            add_dep_helper(first.ins, dep.ins, sync=False,
                                           reason="queue order: after Z")

                # A side: 4 quads of 4 ranges, eager merge + softmax/transpose
                for h in range(4):
                    gsl = slice(h * 8 * K, (h + 1) * 8 * K)
                    for t in range(NR):
                        gather_chunk(
                            ga[t], gsl,
                            bass.AP(rts, t * RW * K,
                                    [[K, V - t * RW], [1, K]]),
                            V - t * RW, K,
                            aidx_sb[:, t * AW + h * 64:t * AW + (h + 1) * 64])
                    for t in range(1, NR):
                        nc.vector.copy_predicated(
                            out=ea[:, gsl],
                            mask=amask_sb[:, (t - 1) * GA * K + h * 8 * K:
                                          (t - 1) * GA * K + (h + 1) * 8 * K],
                            data=ga[t][:, gsl])
                    a_cluster(2 * h, after=(zexp, zcopy, zmm))
                    a_cluster(2 * h + 1)

                # ---- merge Z partials -> zrec = 1/Z ------------------------
                zallT = pp.tile([64, n_cores], F32, tag="zallT")
                nc.sync.dma_start(
                    zallT[:], bass.AP(zall_d.tensor, zall_d[:].offset,
                                      [[1, 64], [64, n_cores]]))
                z64 = pp.tile([64, 1], F32, tag="z64")
                nc.vector.reduce_sum(z64[:], zallT[:], axis=AX.X)
                zrec = pp.tile([64, 1], F32, tag="zrec")
                nc.vector.reciprocal(zrec[:], z64[:])

                # ---- B transposes + ebp = exp * 1/Z (deferred until zrec so
                # the ps_tr ring is never held across the collective) --------
                for c in range(GB // 4):
                    pst = ps_tr.tile([64, 512], BF16, tag="tr")
                    for t in range(4):
                        g = c * 4 + t
                        nc.tensor.transpose(
                            out=pst[:, t * 128:(t + 1) * 128],
                            in_=tsbB[:, g * K:(g + 1) * K],
                            identity=identB[:])
                    sl = slice(c * 512, (c + 1) * 512)
                    nc.scalar.activation(ebp[:, sl], pst[:], AF.Exp)
                    nc.vector.tensor_tensor(
                        out=ebp[:, sl], in0=ebp[:, sl],
                        in1=zrec[:].to_broadcast([64, 512]), op=ALU.mult)

                # ---- main loop: all of saT/ebp is ready early now ----------
                for h in range(2):
                    for m in range(LS // 128):
                        ot = op_.tile([128, 2048], BF16, tag="ot")
                        for half in range(2):
                            pst = ps_mm.tile([128, 1024], F32, tag="mm")
                            for s in range(2):
                                j = h * 4 + half * 2 + s
                                nc.tensor.matmul(
                                    pst[:, s * 512:(s + 1) * 512],
                                    lhsT=ebp[:, m * 128:(m + 1) * 128],
                                    rhs=saT[:, j * 512:(j + 1) * 512],
                                    start=True, stop=True)
                            nc.scalar.activation(
                                ot[:, half * 1024:(half + 1) * 1024], pst[:],
                                AF.Ln, scale=ebias[:, m:m + 1])
                        nc.sync.dma_start(
                            bass.AP(out, m * 128 * RS + h * 2048,
                                    [[RS, 128], [1, 2048]]),
                            ot[:])

    nc.compile()
    return nc


def _wrap16(idx):
    """int16 idx vector -> [128, n/16] tile: j -> [j % 16, j // 16],
    16-row block replicated x8 down the partitions (one per Q7 core)."""
    n = len(idx)
    return np.tile(idx.reshape(n // 16, 16).T, (8, 1)).astype(np.int16)


def make_in_maps(rhs_type_scores, type_lhs_scores, lhs_nonterminal_bias,
                 rhs_emb_idxs, lhs_emb_idxs, v=V, k=K, r=R, n_cores=N_CORES):
    """Host-side input marshalling (layout only): replicate gather tables,
    build the padded transposed tables, slice Z shards, and build the
    per-range clamped int16 index tiles + membership masks."""
    rts_np = np.ascontiguousarray(np.asarray(rhs_type_scores, dtype=np.float32))
    ts_np = np.asarray(type_lhs_scores, dtype=np.float32)
    bias_np = np.asarray(lhs_nonterminal_bias, dtype=np.float32).reshape(V)
    # Z-pass layout (65-wide rows, padded; pad rows exp to ~0)
    tsb65 = np.full((VP, K + 1), -50.0, dtype=np.float32)
    tsb65[:V, :K] = ts_np.T
    tsb65[:V, K] = bias_np
    tsb65[V:, K] = 0.0
    # B-gather layout (128-wide rows for the 256B elem rule)
    tsb2_np = np.zeros((VP, 128), dtype=np.float32)
    tsb2_np[:V, :K] = ts_np.T
    tsb2_np[:V, K] = bias_np
    tsb2_np = np.ascontiguousarray(tsb2_np)
    ridx = np.asarray(rhs_emb_idxs, dtype=np.int64)
    lidx = np.asarray(lhs_emb_idxs, dtype=np.int64)

    def range_idx(idx, mask_rep):
        """Per-range clamped int16 tiles [128, NR*(n/16)] and per-element
        masks [128, (NR-1)*(n/128)*mask_rep] (ranges 1..3)."""
        n = len(idx)
        tiles, masks = [], []
        for t in range(NR):
            hi = min(RW - 1, V - 1 - t * RW)
            tiles.append(_wrap16(np.clip(idx - t * RW, 0, hi)))
            if t > 0:
                m = (idx // RW == t).astype(np.int8)
                masks.append(np.repeat(m.reshape(n // 128, 128).T, mask_rep,
                                       axis=1))
        return (np.ascontiguousarray(np.concatenate(tiles, axis=1)),
                np.ascontiguousarray(np.concatenate(masks, axis=1)))

    in_maps = []
    for c in range(n_cores):
        i, j = divmod(c, PJ)
        a_t, a_m = range_idx(ridx[j * RS:(j + 1) * RS], K)
        b_t, b_m = range_idx(lidx[i * LS:(i + 1) * LS], 128)
        in_maps.append({
            "rts": rts_np, "tsb2": tsb2_np,
            "zts": np.ascontiguousarray(tsb65[c * ZR:(c + 1) * ZR]),
            "aidx": a_t, "bidx": b_t, "amask": a_m, "bmask": b_m,
        })
    return in_maps


def kernel(rhs_type_scores, type_lhs_scores, lhs_nonterminal_bias,
           rhs_emb_idxs, lhs_emb_idxs):
    nc = build()
    in_maps = make_in_maps(rhs_type_scores, type_lhs_scores,
                           lhs_nonterminal_bias, rhs_emb_idxs, lhs_emb_idxs)
    res = run_bass_kernel_spmd(nc, in_maps, core_ids=list(range(N_CORES)))
    full = np.empty((L, R), dtype=np.float32)
    for c in range(N_CORES):
        i, j = divmod(c, PJ)
        full[i * LS:(i + 1) * LS, j * RS:(j + 1) * RS] = np.asarray(
            res.results[c]["out"]).astype(np.float32)
    return full
